# revision 15
# baseline (speedup 1.0000x reference)
"""Trainium2 Bass kernel for a Griffin-style ChimeraBlock:
   pre-norm RG-LRU recurrence branch + pre-norm SwiGLU FFN, B=2, T=2048,
   D=H=2048, FFN=5632, fp32 I/O.

Parallelization over 8 NeuronCores (tensor-parallel), chunk-pair pipelined:
  - recurrence hidden dim H sharded 8x (256/core); the scan is elementwise
    per channel (native DVE tensor_tensor_scan).
  - norm1 stats computed LOCALLY per core from the replicated x (no AR).
  - rec_out computed as PARTIAL sums over the local H-shard, summed with a
    per-pair ReduceScatter over D (inv_rms of hs factored out of the matmul
    and applied post-RS; its tiny AllReduce rides off the critical path).
  - norm2 stats: tiny per-pair AllReduce; h2 AllGathered per pair (bf16).
  - FFN hidden sharded 8x (704 -> padded 768); down-proj partials
    ReduceScattered per chunk.
  All collectives are emitted in pipeline order (the CC queue executes them
  in program order and blocks to completion) so they hide behind PE work.
Matmuls run in bf16 (fp32 PSUM accumulation); scan state f32; residual f32.
Host side only reshapes/transposes/casts/shards and folds the rmsnorm gain
vectors into adjacent weight matrices.
"""

import sys

sys.path.insert(0, "/opt/trn_rl_repo")

import numpy as np
import ml_dtypes

import concourse.bass as bass
from concourse.bass import _add_dep_helper
import concourse.mybir as mybir
import concourse.tile as tile
from concourse import bacc
from concourse.bass_utils import run_bass_kernel_spmd

BF16 = mybir.dt.bfloat16
F32 = mybir.dt.float32
AF = mybir.ActivationFunctionType
OP = mybir.AluOpType

B, T, D = 2, 2048, 2048
H, FFN = 2048, 5632
NC = 8
HS = H // NC          # 256 hidden shard
DS = D // NC          # 256 d-model shard (output sharding)
FS = FFN // NC        # 704 ffn shard
FSP = 768             # ffn shard padded to a multiple of 128 (pad weights = 0)
BT = B * T            # 4096
CH = 512              # time-chunk (columns)
NCH = BT // CH        # 8 chunks
NPAIR = NCH // 2      # 4 chunk-pairs (collective granularity)
CPB = T // CH         # 4 chunks per batch element (scan resets at b boundary)
KD = D // 128         # 16 k-tiles when contracting over D
KF = FSP // 128       # 6 k-tiles when contracting over ffn shard
EPS = 1e-6
CCONST = 8.0

NP_BF16 = ml_dtypes.bfloat16


def _r128(ap):
    # [R, N] dram view -> [128, R//128, N] (partition, k-tile, col)
    return ap.rearrange("(k p) n -> p k n", p=128)


def build_nc():
    nc = bacc.Bacc("TRN2", target_bir_lowering=False, debug=False, num_devices=NC)
    rg = [list(range(NC))]

    # ---------------- kernel I/O (per core) ----------------
    xt = nc.dram_tensor("xt", [D, BT], BF16, kind="ExternalInput")      # x^T replicated
    xres = nc.dram_tensor("xres", [DS, BT], BF16, kind="ExternalInput")  # x^T d-shard
    w3 = nc.dram_tensor("w3", [D, 3 * HS], BF16, kind="ExternalInput")  # in|ig|rg lhsT shard
    wro = nc.dram_tensor("wro", [HS, D], BF16, kind="ExternalInput")    # rec_out lhsT h-shard
    wg = nc.dram_tensor("wg", [D, FSP], BF16, kind="ExternalInput")
    wu = nc.dram_tensor("wu", [D, FSP], BF16, kind="ExternalInput")
    wd = nc.dram_tensor("wd", [FSP, D], BF16, kind="ExternalInput")
    # cols: 0 = rec_lambda, 1 = ig bias, 2 = rg bias, 3 = h0
    smalls = nc.dram_tensor("smalls", [HS, 4], F32, kind="ExternalInput")
    y = nc.dram_tensor("y", [DS, BT], F32, kind="ExternalOutput")

    with tile.TileContext(nc) as tc:
        with (
            tc.tile_pool(name="sb", bufs=2) as sb,
            tc.tile_pool(name="ps", bufs=2, space="PSUM") as ps,
            tc.tile_pool(name="dr", bufs=1, space="DRAM") as dr,
        ):
            build_body(nc, tc, sb, ps, dr, rg,
                       xt, xres, w3, wro, wg, wu, wd, smalls, y)
    nc.compile()
    return nc


def build_body(nc, tc, sb, ps, dr, rg, xt, xres, w3, wro, wg, wu, wd, smalls, y):
    AG = "AllGather"
    AR = "AllReduce"
    RS = "ReduceScatter"
    dma = nc.sync.dma_start

    # ---------------- internal DRAM ----------------
    ar2_in = [dr.tile([1, 2 * CH], F32, name=f"ar2_in{j}") for j in range(NPAIR)]
    ar2_out = [dr.tile([1, 2 * CH], F32, name=f"ar2_out{j}", addr_space="Shared")
               for j in range(NPAIR)]
    ar3_in = [dr.tile([1, 2 * CH], F32, name=f"ar3_in{j}") for j in range(NPAIR)]
    ar3_out = [dr.tile([1, 2 * CH], F32, name=f"ar3_out{j}", addr_space="Shared")
               for j in range(NPAIR)]
    rec_part = [dr.tile([D, CH], BF16, name=f"rec_part{c}") for c in range(NCH)]
    rec_red = [dr.tile([DS, CH], BF16, name=f"rec_red{c}") for c in range(NCH)]
    agin_h2 = [dr.tile([DS, 2 * CH], BF16, name=f"agin_h2{j}") for j in range(NPAIR)]
    agout_h2 = [dr.tile([D, 2 * CH], BF16, name=f"agout_h2{j}", addr_space="Shared")
                for j in range(NPAIR)]
    ffn_part = [dr.tile([D, CH], BF16, name=f"ffn_part{c}") for c in range(NCH)]
    ffn_red = [dr.tile([DS, CH], BF16, name=f"ffn_red{c}")
               for c in range(NCH)]
    xnew_dram = dr.tile([DS, BT], F32, name="xnew_dram")

    # ---------------- constants / small tensors ----------------
    ones_bf = sb.tile([128, 1], BF16, name="ones_bf", tag="ones", bufs=1)
    nc.vector.memset(ones_bf[:], 1.0)

    def const_tile(val, cname):
        t = sb.tile([128, 1], F32, name=cname, tag=cname, bufs=1)
        nc.vector.memset(t[:], val)
        return t

    c_ln8 = const_tile(1e-8, "c_ln8")         # Ln bias
    c_eps = const_tile(EPS, "c_eps")          # rmsnorm eps
    c_1eps = const_tile(1.0 + EPS, "c_1eps")  # 1 + eps for sqrt(1 - a^2 + eps)

    smalls_sb = sb.tile([128, 2, 4], F32, name="smalls_sb", tag="smalls", bufs=1)
    dma(out=smalls_sb[:], in_=smalls[:].rearrange("(a p) c -> p a c", p=128))
    sig_l = sb.tile([128, 2], F32, name="sig_l", tag="sig_l", bufs=1)
    nc.scalar.activation(sig_l[:], smalls_sb[:, :, 0], AF.Sigmoid)
    c8_sb = sb.tile([128, 2], F32, name="c8_sb", tag="c8", bufs=1)
    # C * log(sigmoid(lambda) + 1e-8)
    nc.scalar.activation(c8_sb[:], sig_l[:], AF.Ln, bias=c_ln8[:])
    nc.scalar.activation(c8_sb[:], c8_sb[:], AF.Copy, bias=0.0, scale=CCONST)

    # ---------------- weights (w3/wg/wu/wd share tag; wd reuses w3's slot) ----
    w3_sb = sb.tile([128, KD, 3 * HS], BF16, name="w3_sb", tag="bigw", bufs=3)
    dma(out=w3_sb[:], in_=_r128(w3[:]))
    wro_sb = sb.tile([128, 2, D], BF16, name="wro_sb", tag="wro", bufs=1)
    dma(out=wro_sb[:], in_=_r128(wro[:]))

    def make_invc(arc_src, cname, cidx, scale):
        # inv_rms [1, CH] -> broadcast [128, CH]
        arc = sb.tile([1, CH], F32, name=f"arc{cname}_{cidx}", tag="arc", bufs=1)
        if arc_src.space == bass.MemorySpace.DRAM:
            nc.gpsimd.dma_start(out=arc[:], in_=arc_src)
            nc.scalar.activation(arc[:], arc[:], AF.Sqrt, bias=c_eps[:1, :],
                                 scale=scale)
        else:
            nc.scalar.activation(arc[:], arc_src, AF.Sqrt, bias=c_eps[:1, :],
                                 scale=scale)
        nc.vector.reciprocal_approx_fast(out=arc[:], in_=arc[:])
        invc = sb.tile([128, CH], F32, name=f"invc{cname}_{cidx}", tag="invc",
                       bufs=2)
        nc.gpsimd.partition_broadcast(invc[:], arc[:])
        return invc

    cc_insts = []

    def ccop(kind, op, ins, outs):
        inst = nc.gpsimd.collective_compute(kind, op, replica_groups=rg,
                                            ins=ins, outs=outs)
        if cc_insts:
            _add_dep_helper(inst.ins, cc_insts[-1].ins, sync=False,
                            reason="cc queue order")
        cc_insts.append(inst)
        return inst

    # dummy first collective: absorbs cross-core launch skew + ncfw warmup
    # so the first real collective isn't inflated by it
    dum_in = dr.tile([1, 8], F32, name="dum_in")
    dum_out = dr.tile([1, 8], F32, name="dum_out", addr_space="Shared")
    dum_sb = sb.tile([1, 8], F32, name="dum_sb", tag="dum", bufs=1)
    nc.vector.memset(dum_sb[:], 0.0)
    dma(out=dum_in[:], in_=dum_sb[:])
    ccop("AllReduce", OP.add, [dum_in[:]], [dum_out[:]])

    hst_prev = [None]

    def stats1_load(c):
        """xc DMA + local full-D sumsq -> inv_rms1 (no collective)."""
        cs = slice(c * CH, (c + 1) * CH)
        xc = sb.tile([128, KD, CH], BF16, name=f"xc{c}", tag="stream", bufs=2)
        dma(out=xc[:], in_=_r128(xt[:])[:, :, cs])
        psq1 = ps.tile([1, CH], F32, name=f"psq1_{c}", tag="psq", bufs=2)
        for k in range(KD):
            xsq = sb.tile([128, CH], BF16, name=f"xsq{c}_{k}", tag="xsq", bufs=2)
            nc.vector.tensor_tensor(xsq[:], xc[:, k, :], xc[:, k, :], op=OP.mult)
            nc.tensor.matmul(psq1[:], ones_bf[:], xsq[:],
                             start=(k == 0), stop=(k == KD - 1))
        invc1 = make_invc(psq1[:], "1", c, 1.0 / D)
        return xc, invc1

    def proj_third(c, xc, invc1, zt, p_i):
        if True:
            for m in range(2):
                pst = ps.tile([128, CH], F32, name=f"pp{c}_{p_i}_{m}",
                              tag="mm", bufs=6)
                for k in range(KD):
                    nc.tensor.matmul(
                        pst[:],
                        w3_sb[:, k, p_i * HS + m * 128: p_i * HS + (m + 1) * 128],
                        xc[:, k, :],
                        start=(k == 0), stop=(k == KD - 1),
                    )
                z = sb.tile([128, CH], BF16, name=f"z{c}_{p_i}_{m}",
                            tag=f"z{p_i}", bufs=2)
                nc.vector.tensor_tensor(z[:], pst[:], invc1[:], op=OP.mult)
                zt[(p_i, m)] = z

    def gates_scan(c, zt):
        # batched by activation function to minimize ACT table reloads
        hst = sb.tile([128, 2, CH], BF16, name=f"hst{c}", tag="hs", bufs=2)
        it, rt, at, na = {}, {}, {}, {}
        for m in range(2):
            it[m] = sb.tile([128, CH], BF16, name=f"it{c}_{m}", tag="it", bufs=2)
            nc.scalar.activation(it[m][:], zt[(1, m)][:], AF.Sigmoid,
                                 bias=smalls_sb[:, m, 1:2])
            rt[m] = sb.tile([128, CH], F32, name=f"rt{c}_{m}", tag="rt", bufs=2)
            nc.scalar.activation(rt[m][:], zt[(2, m)][:], AF.Sigmoid,
                                 bias=smalls_sb[:, m, 2:3])
        for m in range(2):
            # la = r * (C * log_a)  (in place over rt)
            nc.vector.tensor_scalar_mul(rt[m][:], rt[m][:], c8_sb[:, m:m + 1])
        for m in range(2):
            at[m] = sb.tile([128, CH], F32, name=f"at{c}_{m}", tag="at", bufs=2)
            nc.scalar.activation(at[m][:], rt[m][:], AF.Exp)
        for m in range(2):
            na[m] = sb.tile([128, CH], F32, name=f"na{c}_{m}", tag="na", bufs=2)
            nc.vector.scalar_tensor_tensor(na[m][:], at[m][:], -1.0, at[m][:],
                                           op0=OP.mult, op1=OP.mult)
        for m in range(2):
            nc.scalar.activation(na[m][:], na[m][:], AF.Sqrt, bias=c_1eps[:])
        for m in range(2):
            zx = zt[(0, m)]
            # g = sq * (i * x_proj)   (build in place over zx)
            nc.vector.tensor_tensor(zx[:], it[m][:], zx[:], op=OP.mult)
            nc.vector.tensor_tensor(zx[:], na[m][:], zx[:], op=OP.mult)
            if c % CPB == 0:
                init = smalls_sb[:, m, 3:4]
            else:
                init = hst_prev[0][:, m, CH - 1:CH]
            nc.vector.tensor_tensor_scan(hst[:, m, :], at[m][:], zx[:], init,
                                         op0=OP.mult, op1=OP.add)
        hst_prev[0] = hst
        return hst

    def stats2_rec(c, hst):
        """hs sumsq partial -> ar2 staging; rec_out partials over H-shard."""
        j, jj = c // 2, c % 2
        hsq = sb.tile([128, 2, CH], BF16, name=f"hsq{c}", tag="hsq", bufs=1)
        nc.vector.tensor_tensor(hsq[:], hst[:], hst[:], op=OP.mult)
        psq2 = ps.tile([1, CH], F32, name=f"psq2_{c}", tag="psq", bufs=2)
        nc.tensor.matmul(psq2[:], ones_bf[:], hsq[:, 0, :], start=True, stop=False)
        nc.tensor.matmul(psq2[:], ones_bf[:], hsq[:, 1, :], start=False, stop=True)
        sqs2 = sb.tile([1, CH], F32, name=f"sqs2_{c}", tag="sqs", bufs=1)
        nc.scalar.copy(sqs2[:], psq2[:])
        nc.scalar.dma_start(out=ar2_in[j][0:1, jj * CH:(jj + 1) * CH], in_=sqs2[:])
        # rec_out partial (raw hs; inv_rms2 applied post-RS)
        for m in range(KD):
            psr = ps.tile([128, CH], F32, name=f"pr{c}_{m}", tag="mm", bufs=6)
            nc.tensor.matmul(psr[:], wro_sb[:, 0, m * 128:(m + 1) * 128],
                             hst[:, 0, :], start=True, stop=False)
            nc.tensor.matmul(psr[:], wro_sb[:, 1, m * 128:(m + 1) * 128],
                             hst[:, 1, :], start=False, stop=True)
            dstr = sb.tile([128, CH], BF16, name=f"dstr{c}_{m}", tag="dst", bufs=3)
            if m % 2 == 0:
                nc.scalar.copy(dstr[:], psr[:])
            else:
                nc.vector.tensor_copy(dstr[:], psr[:])
            dma(out=rec_part[c][m * 128:(m + 1) * 128, :], in_=dstr[:])

    xnew_pairs = {}

    def pp_a(j):
        """xnew = x + inv2*rec_red; stats3 -> AR3."""
        xnew_t = {}
        for jj in range(2):
            c = 2 * j + jj
            cs = slice(c * CH, (c + 1) * CH)
            recr = sb.tile([128, 2, CH], BF16, name=f"recr{c}", tag="recr", bufs=2)
            nc.gpsimd.dma_start(out=recr[:], in_=_r128(rec_red[c][:])[:, :, :])
            xrs = sb.tile([128, 2, CH], BF16, name=f"xrs{c}", tag="xrs", bufs=1)
            dma(out=xrs[:], in_=_r128(xres[:])[:, :, cs])
            invc2 = make_invc(ar2_out[j][0:1, jj * CH:(jj + 1) * CH], "2", c,
                              1.0 / H)
            xnew = sb.tile([128, 2, CH], F32, name=f"xnew{c}", tag="xnew", bufs=2)
            for m in range(2):
                nc.vector.tensor_tensor(xnew[:, m, :], recr[:, m, :], invc2[:],
                                        op=OP.mult)
                nc.vector.tensor_tensor(xnew[:, m, :], xnew[:, m, :],
                                        xrs[:, m, :], op=OP.add)
            xnew_t[jj] = xnew
            dma(out=_r128(xnew_dram[:])[:, :, cs], in_=xnew[:])
            xnq = sb.tile([128, 2, CH], BF16, name=f"xnq{c}", tag="hsq", bufs=1)
            nc.vector.tensor_tensor(xnq[:], xnew[:], xnew[:], op=OP.mult)
            psq3 = ps.tile([1, CH], F32, name=f"psq3_{c}", tag="psq", bufs=2)
            nc.tensor.matmul(psq3[:], ones_bf[:], xnq[:, 0, :], start=True,
                             stop=False)
            nc.tensor.matmul(psq3[:], ones_bf[:], xnq[:, 1, :], start=False,
                             stop=True)
            sqs3 = sb.tile([1, CH], F32, name=f"sqs3_{c}", tag="sqs", bufs=1)
            nc.scalar.copy(sqs3[:], psq3[:])
            nc.scalar.dma_start(out=ar3_in[j][0:1, jj * CH:(jj + 1) * CH],
                                in_=sqs3[:])
        xnew_pairs[j] = xnew_t
        ccop(AR, OP.add, [ar3_in[j][:]], [ar3_out[j][:]])

    def pp_b(j):
        """h2 = xnew*inv3 -> AG."""
        xnew_t = xnew_pairs[j]
        for jj in range(2):
            c = 2 * j + jj
            invc3 = make_invc(ar3_out[j][0:1, jj * CH:(jj + 1) * CH], "3", c,
                              1.0 / D)
            h2t = sb.tile([128, 2, CH], BF16, name=f"h2t{c}", tag="h2t", bufs=2)
            for m in range(2):
                nc.vector.tensor_tensor(h2t[:, m, :], xnew_t[jj][:, m, :],
                                        invc3[:], op=OP.mult)
            nc.scalar.dma_start(
                out=_r128(agin_h2[j][:])[:, :, jj * CH:(jj + 1) * CH], in_=h2t[:])
        ccop(AG, OP.bypass, [agin_h2[j][:]], [agout_h2[j][:]])

    def ffn_chunk(c, wg_sb, wu_sb, wd_sb):
        j, jj = c // 2, c % 2
        h2s = sb.tile([128, KD, CH], BF16, name=f"h2s{c}", tag="stream", bufs=2)
        nc.scalar.dma_start(out=h2s[:],
                            in_=_r128(agout_h2[j][:])[:, :, jj * CH:(jj + 1) * CH])
        gu = sb.tile([128, KF, CH], BF16, name=f"gu{c}", tag="gu", bufs=2)
        for m in range(KF):
            psg = ps.tile([128, CH], F32, name=f"pg{c}_{m}", tag="mm", bufs=6)
            for k in range(KD):
                nc.tensor.matmul(psg[:], wg_sb[:, k, m * 128:(m + 1) * 128],
                                 h2s[:, k, :],
                                 start=(k == 0), stop=(k == KD - 1))
            gs = sb.tile([128, CH], BF16, name=f"gs{c}_{m}", tag="gsil", bufs=1)
            nc.scalar.activation(gs[:], psg[:], AF.Silu)
            psu = ps.tile([128, CH], F32, name=f"pu{c}_{m}", tag="mm", bufs=6)
            for k in range(KD):
                nc.tensor.matmul(psu[:], wu_sb[:, k, m * 128:(m + 1) * 128],
                                 h2s[:, k, :],
                                 start=(k == 0), stop=(k == KD - 1))
            nc.vector.tensor_tensor(gu[:, m, :], psu[:], gs[:], op=OP.mult)
        for m in range(KD):
            psd = ps.tile([128, CH], F32, name=f"pd{c}_{m}", tag="mm", bufs=6)
            for k in range(KF):
                nc.tensor.matmul(psd[:], wd_sb[:, k, m * 128:(m + 1) * 128],
                                 gu[:, k, :],
                                 start=(k == 0), stop=(k == KF - 1))
            dstf = sb.tile([128, CH], BF16, name=f"dstf{c}_{m}", tag="dst", bufs=3)
            nc.vector.tensor_copy(dstf[:], psd[:])
            dma(out=ffn_part[c][m * 128:(m + 1) * 128, :], in_=dstf[:])

    def ffn_rs(c):
        ccop(RS, OP.add, [ffn_part[c][:]], [ffn_red[c][:]])

    def final_chunk(c):
        cs = slice(c * CH, (c + 1) * CH)
        frt = sb.tile([128, 2, CH], BF16, name=f"frt{c}", tag="recr", bufs=2)
        nc.gpsimd.dma_start(out=frt[:], in_=_r128(ffn_red[c][:])[:, :, :])
        xnt = sb.tile([128, 2, CH], F32, name=f"xnt{c}", tag="xnew", bufs=2)
        dma(out=xnt[:], in_=_r128(xnew_dram[:])[:, :, cs])
        for m in range(2):
            nc.vector.tensor_tensor(xnt[:, m, :], xnt[:, m, :], frt[:, m, :],
                                    op=OP.add)
        dma(out=_r128(y[:])[:, :, cs], in_=xnt[:])

    # ---------------- pipelined emission ----------------
    # Emission order IS the per-engine execution order (Tile's schedule is
    # static): all 8 chunks of proj/scan/rec first (stats pipelined one chunk
    # ahead, rec one chunk behind), then the 8 FFN chunks.  Collectives are
    # emitted at their data-readiness points so the single CC queue (program
    # order, run-to-completion) never blocks a consumer.
    stats = {0: stats1_load(0)}
    wg_sb = wu_sb = wd_sb = None
    zts = {}
    hsts = {}
    for c in range(NCH):
        xc, invc1 = stats[c]
        zts[c] = {}
        proj_third(c, xc, invc1, zts[c], 0)
        if c + 1 < NCH:
            stats[c + 1] = stats1_load(c + 1)
        proj_third(c, xc, invc1, zts[c], 1)
        proj_third(c, xc, invc1, zts[c], 2)
        if c == 2:
            wg_sb = sb.tile([128, KD, FSP], BF16, name="wg_sb", tag="bigw", bufs=3)
            dma(out=wg_sb[:], in_=_r128(wg[:]))
            wu_sb = sb.tile([128, KD, FSP], BF16, name="wu_sb", tag="bigw", bufs=3)
            dma(out=wu_sb[:], in_=_r128(wu[:]))
            wd_sb = sb.tile([128, KF, D], BF16, name="wd_sb", tag="wdt", bufs=1)
            dma(out=wd_sb[:], in_=_r128(wd[:]))
        if c == 6:
            pp_a(0)
        if c >= 1:
            stats2_rec(c - 1, hsts[c - 1])
            if (c - 1) % 2 == 1:
                j = (c - 1) // 2
                ccop(AR, OP.add, [ar2_in[j][:]], [ar2_out[j][:]])
            ccop(RS, OP.add, [rec_part[c - 1][:]], [rec_red[c - 1][:]])
        hsts[c] = gates_scan(c, zts[c])
        if c == 7:
            pp_b(0)
    stats2_rec(NCH - 1, hsts[NCH - 1])
    ccop(AR, OP.add, [ar2_in[NPAIR - 1][:]], [ar2_out[NPAIR - 1][:]])
    ccop(RS, OP.add, [rec_part[NCH - 1][:]], [rec_red[NCH - 1][:]])
    pp_a(1)
    ffn_chunk(0, wg_sb, wu_sb, wd_sb)
    pp_b(1)
    ffn_chunk(1, wg_sb, wu_sb, wd_sb)
    pp_a(2)
    ffn_chunk(2, wg_sb, wu_sb, wd_sb)
    ffn_rs(0)
    pp_b(2)
    ffn_chunk(3, wg_sb, wu_sb, wd_sb)
    ffn_rs(1)
    pp_a(3)
    ffn_chunk(4, wg_sb, wu_sb, wd_sb)
    ffn_rs(2)
    pp_b(3)
    ffn_chunk(5, wg_sb, wu_sb, wd_sb)
    ffn_rs(3)
    ffn_chunk(6, wg_sb, wu_sb, wd_sb)
    ffn_rs(4)
    ffn_chunk(7, wg_sb, wu_sb, wd_sb)
    for c in range(5, NCH):
        ffn_rs(c)
    for c in range(NCH):
        final_chunk(c)


_CACHE = {}


def _prep_inputs(inputs):
    f = np.float32
    x = np.asarray(inputs["x"], f)                       # [B, T, D]
    norm1_w = np.asarray(inputs["norm1_w"], f)
    rec_in_w = np.asarray(inputs["rec_in_w"], f)         # [H, D]
    rec_ig_w = np.asarray(inputs["rec_ig_w"], f)
    rec_ig_b = np.asarray(inputs["rec_ig_b"], f)
    rec_rg_w = np.asarray(inputs["rec_rg_w"], f)
    rec_rg_b = np.asarray(inputs["rec_rg_b"], f)
    rec_lambda = np.asarray(inputs["rec_lambda"], f)
    rec_out_w = np.asarray(inputs["rec_out_w"], f)       # [D, H]
    rec_h0 = np.asarray(inputs["rec_h0"], f)             # [1, 1, H]
    rec_norm_w = np.asarray(inputs["rec_norm_w"], f)
    norm2_w = np.asarray(inputs["norm2_w"], f)
    ffn_gate_w = np.asarray(inputs["ffn_gate_w"], f)     # [FFN, D]
    ffn_up_w = np.asarray(inputs["ffn_up_w"], f)
    ffn_down_w = np.asarray(inputs["ffn_down_w"], f)     # [D, FFN]

    xt_full = np.ascontiguousarray(
        x.reshape(BT, D).T.astype(NP_BF16))              # [D, BT]

    # fold norm gains into adjacent weights; transpose into lhsT layouts
    w_in_t = (rec_in_w * norm1_w[None, :]).T             # [D, H]
    w_ig_t = (rec_ig_w * norm1_w[None, :]).T
    w_rg_t = (rec_rg_w * norm1_w[None, :]).T
    w_ro = rec_out_w * rec_norm_w[None, :]               # [D, H]
    w_g_t = (ffn_gate_w * norm2_w[None, :]).T            # [D, FFN]
    w_u_t = (ffn_up_w * norm2_w[None, :]).T
    w_d_t = ffn_down_w.T                                 # [FFN, D]

    in_maps = []
    for r in range(NC):
        hsl = slice(r * HS, (r + 1) * HS)
        dsl = slice(r * DS, (r + 1) * DS)
        fsl = slice(r * FS, (r + 1) * FS)
        w3_r = np.concatenate(
            [w_in_t[:, hsl], w_ig_t[:, hsl], w_rg_t[:, hsl]], axis=1)
        wro_r = np.ascontiguousarray(w_ro.T[hsl, :])     # [HS, D]
        wg_r = np.zeros((D, FSP), f)
        wg_r[:, :FS] = w_g_t[:, fsl]
        wu_r = np.zeros((D, FSP), f)
        wu_r[:, :FS] = w_u_t[:, fsl]
        wd_r = np.zeros((FSP, D), f)
        wd_r[:FS, :] = w_d_t[fsl, :]
        smalls_r = np.stack(
            [rec_lambda[hsl], rec_ig_b[hsl], rec_rg_b[hsl],
             np.broadcast_to(rec_h0[0, 0], (H,))[hsl]], axis=1)
        in_maps.append({
            "xt": xt_full,
            "xres": np.ascontiguousarray(xt_full[dsl, :]),
            "w3": np.ascontiguousarray(w3_r.astype(NP_BF16)),
            "wro": wro_r.astype(NP_BF16),
            "wg": np.ascontiguousarray(wg_r.astype(NP_BF16)),
            "wu": np.ascontiguousarray(wu_r.astype(NP_BF16)),
            "wd": np.ascontiguousarray(wd_r.astype(NP_BF16)),
            "smalls": np.ascontiguousarray(smalls_r.astype(f)),
        })
    return in_maps


def run_on_device(inputs, trace=False, tmpdir=None):
    if "nc" not in _CACHE:
        _CACHE["nc"] = build_nc()
    nc = _CACHE["nc"]
    in_maps = _prep_inputs(inputs)
    res = run_bass_kernel_spmd(nc, in_maps, list(range(NC)),
                               trace=trace, tmpdir=tmpdir)
    shards = [np.asarray(res.results[r]["y"]) for r in range(NC)]
    yt = np.concatenate(shards, axis=0)                  # [D, BT]
    out = np.ascontiguousarray(yt.T).reshape(B, T, D).astype(np.float32)
    return out, res


def kernel(**inputs):
    out, _ = run_on_device(inputs, trace=False)
    return out
